# revision 5
# baseline (speedup 1.0000x reference)
"""Trainium2 Bass kernel for nn_Conv_6511170421767.

3x3 conv, stride 1, pad 1 on x:(32,128,56,56) with weight:(256,128,3,3),
bias:(256,) -> out:(32,256,56,56), fp32 in/out.

Strategy (data-parallel, 4 images per core on 8 cores), 1D Winograd
F(2,3) along the width:
- Cin=128 is the PE contraction/partition dim. For each output column
  pair (2t, 2t+1) the 3 width-taps collapse to 4 transformed products:
    U0 = d0-d2, U1 = d1+d2, U2 = d2-d1, U3 = d1-d3   (d_c = xpad col 2t+c)
    m_nu = sum_dr  Gw[dr,nu]^T @ U_nu[row+dr]        (PSUM, 3 matmuls/nu)
    out_even = m0+m1+m2+b,  out_odd = m1-m2-m3+b
  so PE streaming per (14-row block, cout-chunk) is 12 matmuls of N=392
  instead of the direct 9 of N=784: 150,528 PE cycles/core vs 225,792.
- The height taps stay as PSUM accumulation (dr shifts the U row slice),
  so no vertical transform is needed; U border rows (padded rows 0/57)
  are simply zeroed.
- Transforms are spread across the non-PE engines, all under the
  1.96us/iter PE budget: DVE does U1/U2 + even-column inverse, Pool
  (gpsimd) does U0/U3 + odd-column inverse, Activation folds the bias
  while reading m2/m3 from PSUM. Every tensor_tensor reads at most one
  PSUM operand.
- Matmul operands are fp16 (1 PE cycle/row; operand ranges sit safely
  inside fp16; F(2,3) constants are 0.5/1 so no amplification).
  Accumulation is fp32 in PSUM.
- x is DMA'd as one contiguous [128,56,56] f32 transfer per image (no
  padding copy at all); output DMAs move [128,14,56] blocks whose DRAM
  runs are 3136B contiguous, avoiding the <512B DMA penalty.

The external neuronxcc walrus in this container enforces small per-
instruction sync-wait limits (TRN2 HW allows 1 per instruction). Tile
emits up to ~10 waits on the final drain, so _cap_sync_waits() splits
excess waits onto InstNoOp instructions inserted just before the
offender on the same engine.
"""

import sys

sys.path.insert(0, "/opt/trn_rl_repo")

import numpy as np

import concourse.bass as bass
import concourse.mybir as mybir
import concourse.tile as tile
from concourse.bass_utils import run_bass_kernel_spmd

F32 = mybir.dt.float32
FP16 = mybir.dt.float16
ADD = mybir.AluOpType.add
SUB = mybir.AluOpType.subtract

N_CORES = 8
IMGS_PER_CORE = 4
CIN = 128
COUT = 256
H = W = 56
T = W // 2  # 28 column pairs
ROWS_PER_TILE = 14  # output rows per iter -> N = 14*28 = 392 (one PSUM bank)
N_ROW_TILES = H // ROWS_PER_TILE  # 4
NTILE = ROWS_PER_TILE * T  # 392

_WAIT_LIMITS_DEFAULT = 1
_WAIT_LIMITS = {}


def _cap_sync_waits(nc):
    """Split sync waits exceeding per-instruction limits onto same-engine
    InstNoOp instructions inserted immediately before the offender."""
    for fn in nc.m.functions:
        for bb in fn.blocks:
            i = 0
            insts = bb.instructions
            while i < len(insts):
                inst = insts[i]
                si = getattr(inst, "sync_info", None)
                if si is None or not si.on_wait:
                    i += 1
                    continue
                limit = _WAIT_LIMITS.get(type(inst).__name__, _WAIT_LIMITS_DEFAULT)
                waits = list(si.on_wait)
                if len(waits) <= limit:
                    i += 1
                    continue
                keep = waits[:limit]
                excess = waits[limit:]
                inst.sync_info = mybir.SyncInfo(
                    on_wait=keep, on_update=list(si.on_update)
                )
                pos = i
                for j in range(0, len(excess), _WAIT_LIMITS_DEFAULT):
                    chunk = excess[j : j + _WAIT_LIMITS_DEFAULT]
                    nop = mybir.InstNoOp(
                        name=nc.get_next_instruction_name(), ins=[], outs=[]
                    )
                    nop.engine = inst.engine
                    nop.sync_info = mybir.SyncInfo(on_wait=chunk, on_update=[])
                    nc.register_instruction(nop)
                    insts.insert(pos, nop)
                    pos += 1
                    i += 1
                i += 1


def build_conv_nc():
    """One-core program: x:(4,128,56,56), wT:(128,12,256) transformed
    weights, bias2:(128,2) -> out:(4,256,56,56)."""
    nc = bass.Bass()
    x = nc.dram_tensor("x", [IMGS_PER_CORE, CIN, H, W], F32, kind="ExternalInput")
    wt = nc.dram_tensor("wT", [CIN, 12, COUT], F32, kind="ExternalInput")
    bias2 = nc.dram_tensor("bias2", [128, 2], F32, kind="ExternalInput")
    out = nc.dram_tensor(
        "out", [IMGS_PER_CORE, COUT, H, W], F32, kind="ExternalOutput"
    )

    with tile.TileContext(nc) as tc:
        with (
            tc.tile_pool(name="const", bufs=1) as const_pool,
            tc.tile_pool(name="xs", bufs=2) as xs_pool,
            tc.tile_pool(name="uplanes", bufs=2) as u_pool,
            tc.tile_pool(name="post", bufs=3) as post_pool,
            tc.tile_pool(name="obuf", bufs=4) as obuf_pool,
            tc.tile_pool(name="psum", bufs=2, space="PSUM") as psum_pool,
        ):
            # Transformed weights: HWDGE DMA per (dr,nu) tap into an f32
            # stage, DVE-round into fp16. Per-tap split lets the first
            # matmul start early.
            w_stage = const_pool.tile([CIN, 12, COUT], F32)
            w_sb = const_pool.tile([CIN, 12 * COUT], FP16)

            def w_tap(k):
                nc.sync.dma_start(w_stage[:, k, :], wt[:, k, :])
                nc.vector.tensor_copy(
                    w_sb[:, k * COUT : (k + 1) * COUT], w_stage[:, k, :]
                )

            def lhsT(dr, nu, c):
                k = dr * 4 + nu
                return w_sb[:, k * COUT + c * 128 : k * COUT + c * 128 + 128]

            xstages = [
                xs_pool.tile([CIN, H, W], F32, tag="xs", name=f"xs{i}")
                for i in range(2)
            ]
            # U planes: rows 0..57 are padded coords (row 0 and 57 zero).
            uplanes = [
                [
                    u_pool.tile(
                        [CIN, H + 2, T], FP16, tag=f"u{nu}", name=f"u{nu}_{i}"
                    )
                    for nu in range(4)
                ]
                for i in range(2)
            ]

            def x_dma(img):
                nc.scalar.dma_start(xstages[img % 2][:], x[img])

            def u_transform(img):
                xs = xstages[img % 2]
                u = uplanes[img % 2]
                d0 = xs[:, :, 1:54:2]  # cols 1,3..53 (t=1..27 -> 2t-1)
                d1 = xs[:, :, 0:55:2]  # cols 0,2..54  (t=0..27 -> 2t)
                d2 = xs[:, :, 1:56:2]  # cols 1,3..55
                d3 = xs[:, :, 2:55:2]  # cols 2,4..54  (t=0..26 -> 2t+2)
                # border rows (padded rows 0, 57) zero; recompute per img
                # since pool buffers rotate.
                for nu in range(4):
                    nc.gpsimd.memset(u[nu][:, 0, :], 0.0)
                    nc.gpsimd.memset(u[nu][:, H + 1, :], 0.0)
                # interior rows 1..56 <- x rows 0..55 (gpsimd: SBUF-only)
                nc.gpsimd.tensor_tensor(u[0][:, 1 : H + 1, 1:], d0, d2[:, :, 1:], SUB)
                nc.gpsimd.tensor_scalar_mul(u[0][:, 1 : H + 1, 0], xs[:, :, 1], -1.0)
                nc.gpsimd.tensor_tensor(u[1][:, 1 : H + 1, :], d1, d2, ADD)
                nc.gpsimd.tensor_tensor(u[2][:, 1 : H + 1, :], d2, d1, SUB)
                nc.gpsimd.tensor_tensor(u[3][:, 1 : H + 1, 0:27], d1[:, :, 0:27], d3, SUB)
                nc.gpsimd.tensor_copy(u[3][:, 1 : H + 1, 27], xs[:, :, 54])

            # Startup: first image's DMA + weight taps + transforms.
            x_dma(0)
            for k in range(12):
                w_tap(k)
            b_sb = const_pool.tile([128, 2], F32)
            nc.sync.dma_start(b_sb[:], bias2[:])
            u_transform(0)

            for img in range(IMGS_PER_CORE):
                u = uplanes[img % 2]
                if img + 1 < IMGS_PER_CORE:
                    x_dma(img + 1)

                for t in range(N_ROW_TILES):
                    # stage next image's transforms mid-way through this
                    # image so its U planes are ready when needed.
                    if t == 2 and img + 1 < IMGS_PER_CORE:
                        u_transform(img + 1)
                    y0 = t * ROWS_PER_TILE
                    for c in range(2):  # Cout chunks of 128
                        ps = [
                            psum_pool.tile(
                                [128, ROWS_PER_TILE, T],
                                F32,
                                tag=f"ps{nu}",
                                name=f"ps{nu}_{img}_{t}_{c}",
                            )
                            for nu in range(4)
                        ]
                        for nu in range(4):
                            for dr in range(3):
                                nc.tensor.matmul(
                                    ps[nu][:],
                                    lhsT(dr, nu, c),
                                    u[nu][:, y0 + dr : y0 + dr + ROWS_PER_TILE, :],
                                    start=(dr == 0),
                                    stop=(dr == 2),
                                )
                        # inverse transform + bias. Each PSUM bank is read
                        # exactly once (gpsimd cannot touch PSUM):
                        #   scalar: s1 = m1 + b,  e2 = m2
                        #   gpsimd: g1 = s1 + e2, g2 = s1 - e2  (SBUF only)
                        #   DVE:    even = m0 + g1, odd = g2 - m3
                        s1 = post_pool.tile(
                            [128, NTILE], F32, tag="s1", name=f"s1_{img}_{t}_{c}"
                        )
                        e2 = post_pool.tile(
                            [128, NTILE], F32, tag="e2", name=f"e2_{img}_{t}_{c}"
                        )
                        g1 = post_pool.tile(
                            [128, NTILE], F32, tag="g1", name=f"g1_{img}_{t}_{c}"
                        )
                        g2 = post_pool.tile(
                            [128, NTILE], F32, tag="g2", name=f"g2_{img}_{t}_{c}"
                        )
                        ob = obuf_pool.tile(
                            [128, ROWS_PER_TILE, W],
                            F32,
                            tag="ob",
                            name=f"ob_{img}_{t}_{c}",
                        )
                        ps0f = ps[0][:].rearrange("p r t -> p (r t)")
                        ps1f = ps[1][:].rearrange("p r t -> p (r t)")
                        ps2f = ps[2][:].rearrange("p r t -> p (r t)")
                        ps3f = ps[3][:].rearrange("p r t -> p (r t)")
                        nc.scalar.activation(
                            s1[:], ps1f,
                            mybir.ActivationFunctionType.Identity,
                            bias=b_sb[:, c : c + 1], scale=1.0,
                        )
                        nc.scalar.activation(
                            e2[:], ps2f,
                            mybir.ActivationFunctionType.Copy,
                        )
                        nc.gpsimd.tensor_tensor(g1[:], s1[:], e2[:], ADD)
                        nc.gpsimd.tensor_tensor(g2[:], s1[:], e2[:], SUB)
                        obe = ob[:].rearrange("p r (t two) -> p (r t) two", two=2)
                        nc.vector.tensor_tensor(obe[:, :, 0], ps0f, g1[:], ADD)
                        nc.vector.tensor_tensor(obe[:, :, 1], g2[:], ps3f, SUB)
                        nc.sync.dma_start(
                            out[
                                img,
                                c * 128 : (c + 1) * 128,
                                y0 : y0 + ROWS_PER_TILE,
                                :,
                            ],
                            ob[:],
                        )

    _cap_sync_waits(nc)
    nc.finalize()
    return nc


_NC_CACHE = {}


def _get_nc():
    if "nc" not in _NC_CACHE:
        _NC_CACHE["nc"] = build_conv_nc()
    return _NC_CACHE["nc"]


def _prep_in_maps(x, weight, bias):
    x = np.ascontiguousarray(x, dtype=np.float32)
    w = np.asarray(weight, dtype=np.float64)  # (256,128,3,3)
    # Winograd F(2,3) weight transform along the width taps:
    # wtil[dr, nu, ci, co]; nu in {w0, (w0+w1+w2)/2, (w0-w1+w2)/2, w2}
    w0, w1, w2 = w[:, :, :, 0], w[:, :, :, 1], w[:, :, :, 2]
    wtil = np.stack(
        [w0, (w0 + w1 + w2) * 0.5, (w0 - w1 + w2) * 0.5, w2], axis=3
    )  # (co, ci, dr, nu)
    # -> wT[ci, dr*4+nu, co]
    wT = np.ascontiguousarray(
        wtil.transpose(1, 2, 3, 0).reshape(CIN, 12, COUT).astype(np.float32)
    )
    bias2 = np.ascontiguousarray(
        np.asarray(bias, dtype=np.float32).reshape(2, 128).T
    )
    per_core = x.shape[0] // N_CORES
    return [
        {
            "x": x[i * per_core : (i + 1) * per_core],
            "wT": wT,
            "bias2": bias2,
        }
        for i in range(N_CORES)
    ]


def run(x, weight, bias, trace=False):
    """Run the conv on 8 cores; returns (out, BassKernelResults)."""
    nc = _get_nc()
    in_maps = _prep_in_maps(x, weight, bias)
    res = run_bass_kernel_spmd(
        nc, in_maps, core_ids=list(range(N_CORES)), trace=trace
    )
    out = np.concatenate([r["out"] for r in res.results], axis=0)
    return out, res


def kernel(x, weight, bias):
    out, _ = run(x, weight, bias, trace=False)
    return out


# revision 6
# speedup vs baseline: 1.1500x; 1.1500x over previous
"""Trainium2 Bass kernel for nn_Conv_6511170421767.

3x3 conv, stride 1, pad 1 on x:(32,128,56,56) with weight:(256,128,3,3),
bias:(256,) -> out:(32,256,56,56), fp32 in/out.

Strategy (data-parallel, 4 images per core on 8 cores), 1D Winograd
F(2,3) along the width:
- Cin=128 is the PE contraction/partition dim. For each output column
  pair (2t, 2t+1) the 3 width-taps collapse to 4 transformed products:
    U0 = d0-d2, U1 = d1+d2, U2 = d2-d1, U3 = d1-d3   (d_c = xpad col 2t+c)
    m_nu = sum_dr  Gw[dr,nu]^T @ U_nu[row+dr]        (PSUM, 3 matmuls/nu)
    out_even = m0+m1+m2+b,  out_odd = m1-m2-m3+b
  so PE streaming per (14-row block, cout-chunk) is 12 matmuls of N=392
  instead of the direct 9 of N=784: 150,528 PE cycles/core vs 225,792.
- The height taps stay as PSUM accumulation (dr shifts the U row slice);
  U border rows (padded rows 0/57) are just zeroed - no padding copies.
- Measured engine rates (perfetto, this container): DVE tensor_tensor
  ~565ns/392-elem op (PSUM-in), scalar activation ~520-585ns, gpsimd
  tensor_tensor ~1.2us ( slow DSP - give it exactly one op per iter).
  Inverse split: scalar s1=m1+b, e2=m2, s0=m0 (PSUM reads); gpsimd
  g1=s1+e2; DVE even=s0+g1, g2=s1-e2, odd=g2-m3(PSUM). U transform:
  DVE 3 planes, gpsimd 1 plane + edge fixups.
- Weights are Winograd-transformed AND fp16-rounded on the host; they
  DMA straight into SBUF (no on-chip casts). Matmuls are fp16 (1 PE
  cycle/row), fp32 PSUM accumulate. rel err ~4e-4 vs fp32 reference.
- x is DMA'd as contiguous [128,56,56] f32 per image (img0 in two
  halves so the first matmul starts early); output DMAs move
  [128,14,56] blocks whose DRAM runs are 3136B contiguous.

The external neuronxcc walrus in this container enforces small per-
instruction sync-wait limits (TRN2 HW allows 1 per instruction). Tile
emits up to ~10 waits on the final drain, so _cap_sync_waits() splits
excess waits onto InstNoOp instructions inserted just before the
offender on the same engine.
"""

import sys

sys.path.insert(0, "/opt/trn_rl_repo")

import numpy as np

import concourse.bass as bass
import concourse.mybir as mybir
import concourse.tile as tile
from concourse.bass_utils import run_bass_kernel_spmd

F32 = mybir.dt.float32
FP16 = mybir.dt.float16
ADD = mybir.AluOpType.add
SUB = mybir.AluOpType.subtract
IDENT = mybir.ActivationFunctionType.Identity
COPY = mybir.ActivationFunctionType.Copy

N_CORES = 8
IMGS_PER_CORE = 4
CIN = 128
COUT = 256
H = W = 56
T = W // 2  # 28 column pairs
ROWS_PER_TILE = 14  # output rows per iter -> N = 14*28 = 392 (one PSUM bank)
N_ROW_TILES = H // ROWS_PER_TILE  # 4
NTILE = ROWS_PER_TILE * T  # 392

_WAIT_LIMITS_DEFAULT = 1
_WAIT_LIMITS = {}


def _cap_sync_waits(nc):
    """Split sync waits exceeding per-instruction limits onto same-engine
    InstNoOp instructions inserted immediately before the offender."""
    for fn in nc.m.functions:
        for bb in fn.blocks:
            i = 0
            insts = bb.instructions
            while i < len(insts):
                inst = insts[i]
                si = getattr(inst, "sync_info", None)
                if si is None or not si.on_wait:
                    i += 1
                    continue
                limit = _WAIT_LIMITS.get(type(inst).__name__, _WAIT_LIMITS_DEFAULT)
                waits = list(si.on_wait)
                if len(waits) <= limit:
                    i += 1
                    continue
                keep = waits[:limit]
                excess = waits[limit:]
                inst.sync_info = mybir.SyncInfo(
                    on_wait=keep, on_update=list(si.on_update)
                )
                pos = i
                for j in range(0, len(excess), _WAIT_LIMITS_DEFAULT):
                    chunk = excess[j : j + _WAIT_LIMITS_DEFAULT]
                    nop = mybir.InstNoOp(
                        name=nc.get_next_instruction_name(), ins=[], outs=[]
                    )
                    nop.engine = inst.engine
                    nop.sync_info = mybir.SyncInfo(on_wait=chunk, on_update=[])
                    nc.register_instruction(nop)
                    insts.insert(pos, nop)
                    pos += 1
                    i += 1
                i += 1


def build_conv_nc():
    """One-core program: x:(4,128,56,56), wT:(128,12,256) fp16 transformed
    weights, bias2:(128,2) -> out:(4,256,56,56)."""
    nc = bass.Bass()
    x = nc.dram_tensor("x", [IMGS_PER_CORE, CIN, H, W], F32, kind="ExternalInput")
    wt = nc.dram_tensor("wT", [CIN, 12, COUT], FP16, kind="ExternalInput")
    bias2 = nc.dram_tensor("bias2", [128, 2], F32, kind="ExternalInput")
    out = nc.dram_tensor(
        "out", [IMGS_PER_CORE, COUT, H, W], F32, kind="ExternalOutput"
    )

    with tile.TileContext(nc) as tc:
        with (
            tc.tile_pool(name="const", bufs=1) as const_pool,
            tc.tile_pool(name="xs", bufs=2) as xs_pool,
            tc.tile_pool(name="uplanes", bufs=2) as u_pool,
            tc.tile_pool(name="post", bufs=3) as post_pool,
            tc.tile_pool(name="obuf", bufs=4) as obuf_pool,
            tc.tile_pool(name="psum", bufs=2, space="PSUM") as psum_pool,
        ):
            w_sb = const_pool.tile([CIN, 12 * COUT], FP16)
            b_sb = const_pool.tile([128, 2], F32)

            def lhsT(dr, nu, c):
                k = dr * 4 + nu
                return w_sb[:, k * COUT + c * 128 : k * COUT + c * 128 + 128]

            xstages = [
                xs_pool.tile([CIN, H, W], F32, tag="xs", name=f"xs{i}")
                for i in range(2)
            ]
            # U planes: rows 0..57 are padded coords (row 0 and 57 zero).
            uplanes = [
                [
                    u_pool.tile(
                        [CIN, H + 2, T], FP16, tag=f"u{nu}", name=f"u{nu}_{i}"
                    )
                    for nu in range(4)
                ]
                for i in range(2)
            ]

            def x_dma(img, half=None):
                xs = xstages[img % 2]
                if half is None:
                    nc.scalar.dma_start(xs[:], x[img])
                else:
                    r0, r1 = (0, 28) if half == 0 else (28, 56)
                    nc.scalar.dma_start(xs[:, r0:r1, :], x[img, :, r0:r1, :])

            def u_borders(img):
                u = uplanes[img % 2]
                for nu in range(4):
                    nc.gpsimd.memset(u[nu][:, 0, :], 0.0)
                    nc.gpsimd.memset(u[nu][:, H + 1, :], 0.0)

            def u_transform(img, half=None):
                """U interior rows 1..56 <- x rows 0..55 (or a half)."""
                xs = xstages[img % 2]
                u = uplanes[img % 2]
                r0, r1 = (0, H) if half is None else ((0, 28) if half == 0 else (28, H))
                xh = xs[:, r0:r1, :]
                d0 = xh[:, :, 1:54:2]  # cols 1,3..53 (t=1..27 -> 2t-1)
                d1 = xh[:, :, 0:55:2]  # cols 0,2..54  (t=0..27 -> 2t)
                d2 = xh[:, :, 1:56:2]  # cols 1,3..55
                d3 = xh[:, :, 2:55:2]  # cols 2,4..54  (t=0..26 -> 2t+2)
                s = slice(r0 + 1, r1 + 1)
                nc.vector.tensor_tensor(u[0][:, s, 1:], d0, d2[:, :, 1:], SUB)
                nc.vector.tensor_tensor(u[1][:, s, :], d1, d2, ADD)
                nc.vector.tensor_tensor(u[2][:, s, :], d2, d1, SUB)
                nc.gpsimd.tensor_tensor(u[3][:, s, 0:27], d1[:, :, 0:27], d3, SUB)
                nc.gpsimd.tensor_scalar_mul(u[0][:, s, 0], xh[:, :, 1], -1.0)
                nc.gpsimd.tensor_copy(u[3][:, s, 27], xh[:, :, 54])

            # Startup: weights + bias + first image (in halves, so the
            # first row-block's matmuls start as soon as half 0 lands).
            x_dma(0, half=0)
            for k in range(0, 12, 4):  # 3 DMAs of 4 taps (2KB each)
                nc.sync.dma_start(
                    w_sb[:, k * COUT : (k + 4) * COUT], wt[:, k : k + 4, :]
                )
            nc.sync.dma_start(b_sb[:], bias2[:])
            x_dma(0, half=1)
            u_borders(0)
            u_transform(0, half=0)

            for img in range(IMGS_PER_CORE):
                u = uplanes[img % 2]
                if img + 1 < IMGS_PER_CORE:
                    x_dma(img + 1)

                for t in range(N_ROW_TILES):
                    if img == 0 and t == 1:
                        u_transform(0, half=1)
                    if t == 2 and img + 1 < IMGS_PER_CORE:
                        u_borders(img + 1)
                        u_transform(img + 1)
                    y0 = t * ROWS_PER_TILE
                    for c in range(2):  # Cout chunks of 128
                        ps = [
                            psum_pool.tile(
                                [128, ROWS_PER_TILE, T],
                                F32,
                                tag=f"ps{nu}",
                                name=f"ps{nu}_{img}_{t}_{c}",
                            )
                            for nu in range(4)
                        ]
                        for nu in range(4):
                            for dr in range(3):
                                nc.tensor.matmul(
                                    ps[nu][:],
                                    lhsT(dr, nu, c),
                                    u[nu][:, y0 + dr : y0 + dr + ROWS_PER_TILE, :],
                                    start=(dr == 0),
                                    stop=(dr == 2),
                                )
                        # inverse transform + bias:
                        #   scalar: s1 = m1 + b, e2 = m2, s0 = m0
                        #   gpsimd: g1 = s1 + e2
                        #   DVE: even = s0 + g1, g2 = s1 - e2, odd = g2 - m3
                        s1 = post_pool.tile(
                            [128, NTILE], F32, tag="s1", name=f"s1_{img}_{t}_{c}"
                        )
                        e2 = post_pool.tile(
                            [128, NTILE], F32, tag="e2", name=f"e2_{img}_{t}_{c}"
                        )
                        s0 = post_pool.tile(
                            [128, NTILE], F32, tag="s0", name=f"s0_{img}_{t}_{c}"
                        )
                        g1 = post_pool.tile(
                            [128, NTILE], F32, tag="g1", name=f"g1_{img}_{t}_{c}"
                        )
                        g2 = post_pool.tile(
                            [128, NTILE], F32, tag="g2", name=f"g2_{img}_{t}_{c}"
                        )
                        ob = obuf_pool.tile(
                            [128, ROWS_PER_TILE, W],
                            F32,
                            tag="ob",
                            name=f"ob_{img}_{t}_{c}",
                        )
                        ps0f = ps[0][:].rearrange("p r t -> p (r t)")
                        ps1f = ps[1][:].rearrange("p r t -> p (r t)")
                        ps2f = ps[2][:].rearrange("p r t -> p (r t)")
                        ps3f = ps[3][:].rearrange("p r t -> p (r t)")
                        nc.scalar.activation(
                            s1[:], ps1f, IDENT,
                            bias=b_sb[:, c : c + 1], scale=1.0,
                        )
                        nc.scalar.activation(e2[:], ps2f, COPY)
                        nc.scalar.activation(s0[:], ps0f, COPY)
                        nc.gpsimd.tensor_tensor(g1[:], s1[:], e2[:], ADD)
                        obe = ob[:].rearrange("p r (t two) -> p (r t) two", two=2)
                        nc.vector.tensor_tensor(obe[:, :, 0], s0[:], g1[:], ADD)
                        nc.vector.tensor_tensor(g2[:], s1[:], e2[:], SUB)
                        nc.vector.tensor_tensor(obe[:, :, 1], g2[:], ps3f, SUB)
                        nc.sync.dma_start(
                            out[
                                img,
                                c * 128 : (c + 1) * 128,
                                y0 : y0 + ROWS_PER_TILE,
                                :,
                            ],
                            ob[:],
                        )

    _cap_sync_waits(nc)
    nc.finalize()
    return nc


_NC_CACHE = {}


def _get_nc():
    if "nc" not in _NC_CACHE:
        _NC_CACHE["nc"] = build_conv_nc()
    return _NC_CACHE["nc"]


def _prep_in_maps(x, weight, bias):
    x = np.ascontiguousarray(x, dtype=np.float32)
    w = np.asarray(weight, dtype=np.float64)  # (256,128,3,3)
    # Winograd F(2,3) weight transform along the width taps:
    # wtil[dr, nu, ci, co]; nu in {w0, (w0+w1+w2)/2, (w0-w1+w2)/2, w2}
    w0, w1, w2 = w[:, :, :, 0], w[:, :, :, 1], w[:, :, :, 2]
    wtil = np.stack(
        [w0, (w0 + w1 + w2) * 0.5, (w0 - w1 + w2) * 0.5, w2], axis=3
    )  # (co, ci, dr, nu)
    wT = np.ascontiguousarray(
        wtil.transpose(1, 2, 3, 0).reshape(CIN, 12, COUT).astype(np.float16)
    )
    bias2 = np.ascontiguousarray(
        np.asarray(bias, dtype=np.float32).reshape(2, 128).T
    )
    per_core = x.shape[0] // N_CORES
    return [
        {
            "x": x[i * per_core : (i + 1) * per_core],
            "wT": wT,
            "bias2": bias2,
        }
        for i in range(N_CORES)
    ]


def run(x, weight, bias, trace=False):
    """Run the conv on 8 cores; returns (out, BassKernelResults)."""
    nc = _get_nc()
    in_maps = _prep_in_maps(x, weight, bias)
    res = run_bass_kernel_spmd(
        nc, in_maps, core_ids=list(range(N_CORES)), trace=trace
    )
    out = np.concatenate([r["out"] for r in res.results], axis=0)
    return out, res


def kernel(x, weight, bias):
    out, _ = run(x, weight, bias, trace=False)
    return out
